# revision 6
# baseline (speedup 1.0000x reference)
"""Multi-head causal attention (B=8, T=2048, C=384, H=6, Dh=64) on 8 TRN2 cores.

Sharding: data-parallel over batch — core b computes batch element b end to end
(no collectives).

fp8 (e4m3) DoubleRow is used for the Q/K projections and the S = K^T Q
matmuls (2x PE throughput); V/PV/out-proj stay bf16 for accuracy.

Layouts:
  xT8  [64, 3, 2, T]   fp8   c = 128*ci + 64*i + p
  wq8/wk8 [64,3,2,384] fp8   packed cols m (see _pack_qk8)
  wv   [128, 3, 384]   bf16  (c on partitions)
  xT   [128, 3, 2048]  bf16  (for V projection)
  wp   [128, 3, 384]   bf16
  Q8/K8 page0 [128, 2, T] fp8: partition 32a+r = head a, d = 32i+r (i free)
        page1 [64, 2, T]  fp8: heads 4+a
  S matmul per head: DoubleRow lhsT=K8[32a:32a+32, :, chunk] (K = d = 64)
  V_aug [s, 65] per (s-chunk, head), last col = 1; O row 64 = softmax denom
  denom rows bf16-transposed to dT [t%128, h*4+q]; rT = 1/dT per block
  out-proj of block j-1 interleaved into block j's head loop; Y = U*rT + bias
"""

import numpy as np
import ml_dtypes

import concourse.bass as bass
import concourse.tile as tile
from concourse import bacc, mybir
from concourse.bass import ts, ds

F32 = mybir.dt.float32
BF16 = mybir.dt.bfloat16
FP8 = mybir.dt.float8e4
AF = mybir.ActivationFunctionType
DR = mybir.MatmulPerfMode.DoubleRow

B, T, C = 8, 2048, 384
H, DH = 6, 64
SCALE = DH ** -0.5
NCORES = 8
TJ = 512            # q-block width
NJ = T // TJ        # 4 q-blocks
SC = 128            # s-chunk
NCI = C // 128      # 3 channel chunks
NCH = TJ // SC      # s-chunks per q-block (4)


def build_kernel():
    nc = bacc.Bacc("TRN2", target_bir_lowering=False, debug=False)

    xT_d = nc.dram_tensor("xT", [128, NCI, T], BF16, kind="ExternalInput").ap()
    xT8_d = nc.dram_tensor("xT8", [64, NCI, 2, T], FP8, kind="ExternalInput").ap()
    wq8_d = nc.dram_tensor("wq8", [64, NCI, 2, C], FP8, kind="ExternalInput").ap()
    wk8_d = nc.dram_tensor("wk8", [64, NCI, 2, C], FP8, kind="ExternalInput").ap()
    wv_d = nc.dram_tensor("wv", [128, NCI, C], BF16, kind="ExternalInput").ap()
    wp_d = nc.dram_tensor("wp", [128, NCI, C], BF16, kind="ExternalInput").ap()
    biasb_d = nc.dram_tensor("biasb", [128, 384], F32, kind="ExternalInput").ap()
    iden_d = nc.dram_tensor("iden", [1, 1], F32, kind="ExternalInput").ap()
    y_d = nc.dram_tensor("y", [T, C], F32, kind="ExternalOutput").ap()

    with tile.TileContext(nc) as tc:
        with tc.tile_pool(name="const", bufs=1) as cpool:
            xT = cpool.tile([128, NCI, T], BF16)
            xT8 = cpool.tile([64, NCI, 2, T], FP8)
            wq8 = cpool.tile([64, NCI, 2, C], FP8)
            wk8 = cpool.tile([64, NCI, 2, C], FP8)
            wv = cpool.tile([128, NCI, C], BF16)
            wp = cpool.tile([128, NCI, C], BF16)
            biasb = cpool.tile([128, 384], F32)
            iden = cpool.tile([1, 1], F32)
            Q8a = cpool.tile([96, 2, T], FP8)   # heads 0-2
            Q8b = cpool.tile([96, 2, T], FP8)    # heads 3-5
            K8a = cpool.tile([96, 2, T], FP8)
            K8b = cpool.tile([96, 2, T], FP8)
            attT = cpool.tile([128, NCI, T], BF16)
            Vt = cpool.tile([128, 16, H, 65], BF16)

            for ci in range(NCI):
                nc.sync.dma_start(xT[:, ci, :], xT_d[:, ci, :])
                nc.sync.dma_start(xT8[:, ci, :, :], xT8_d[:, ci, :, :])
            nc.sync.dma_start(wq8[:], wq8_d[:])
            nc.sync.dma_start(wk8[:], wk8_d[:])
            nc.sync.dma_start(wv[:], wv_d[:])
            nc.sync.dma_start(wp[:], wp_d[:])
            nc.sync.dma_start(biasb[:], biasb_d[:])
            nc.sync.dma_start(iden[:], iden_d[:])
            # whole-tile memset (contiguous; strided memset fails ISA check);
            # V copies below overwrite cols 0:64, leaving col 64 == 1.0
            nc.gpsimd.memset(Vt[:], 1.0)

            # ---- phase 1: projections ----
            with tc.tile_pool(name="pqk", bufs=3, space="PSUM") as pqk, \
                 tc.tile_pool(name="pv", bufs=2, space="PSUM") as pvp:
                # Q/K in fp8 DoubleRow. Packed cols m: page pg (3 heads),
                # half i: cols [192*pg + 96*i, +96) -> page tile [:, i, :]
                for w8, pga, pgb in ((wq8, Q8a, Q8b), (wk8, K8a, K8b)):
                    for tcn in range(T // 512):
                        for pg, dst8 in ((0, pga), (1, pgb)):
                            for i2 in range(2):
                                ps = pqk.tile([96, 512], F32, tag="pqk")
                                for ci in range(NCI):
                                    nc.tensor.matmul(
                                        ps[:],
                                        lhsT=w8[:, ci, :,
                                                ds(192 * pg + 96 * i2, 96)],
                                        rhs=xT8[:, ci, :, ts(tcn, 512)],
                                        start=(ci == 0), stop=(ci == NCI - 1),
                                        perf_mode=DR,
                                    )
                                nc.vector.tensor_copy(
                                    dst8[:, i2, ts(tcn, 512)], ps[:])
                for si in range(16):
                    ps = pvp.tile([128, C], F32, tag="pv")
                    for ci in range(NCI):
                        nc.tensor.matmul(
                            ps[:],
                            lhsT=xT[:, ci, ts(si, 128)],
                            rhs=wv[:, ci, :],
                            start=(ci == 0), stop=(ci == NCI - 1),
                        )
                    nc.vector.tensor_copy(
                        Vt[:, si, :, 0:64],
                        ps[:].rearrange("p (h d) -> p h d", h=H),
                    )

            # ---- phase 2+3: attention + output projection ----
            with tc.tile_pool(name="sps", bufs=3, space="PSUM") as sps, \
                 tc.tile_pool(name="ops", bufs=2, space="PSUM") as ops, \
                 tc.tile_pool(name="dps", bufs=1, space="PSUM") as dps, \
                 tc.tile_pool(name="ups", bufs=2, space="PSUM") as ups, \
                 tc.tile_pool(name="pp", bufs=3) as pp, \
                 tc.tile_pool(name="rp", bufs=2) as rp, \
                 tc.tile_pool(name="yp", bufs=2) as yp:
                rTs = {}

                def out_proj(jj, q):
                    # Y[t, e] for t-chunk (jj, q): sum over heads of
                    # (attT_h^T @ wp_h) * recip_h[t], plus bias.
                    tb = NCH * jj + q
                    rT = rTs[jj]
                    Y = yp.tile([128, C], F32, tag="Y")
                    for h in range(H):
                        po = (h % 2) * 64
                        bi = h // 2
                        U = ups.tile([128, C], F32, tag="U")
                        nc.tensor.matmul(
                            U[:],
                            lhsT=attT[po:po + 64, bi, ts(tb, 128)],
                            rhs=wp[po:po + 64, bi, :],
                            start=True, stop=True,
                        )
                        sc = rT[:, h * NCH + q:h * NCH + q + 1]
                        nc.vector.scalar_tensor_tensor(
                            out=Y[:], in0=U[:], scalar=sc,
                            in1=(biasb[:] if h == 0 else Y[:]),
                            op0=mybir.AluOpType.mult,
                            op1=mybir.AluOpType.add,
                        )
                    nc.sync.dma_start(y_d[ts(tb, 128), :], Y[:])

                for j in range(NJ):
                    # denominators of all 6 heads, transposed: dT[t%128, h*4+q]
                    dT = dps.tile([128, NCH * H], F32, tag="dT")
                    for h in range(H):
                        Q8 = Q8a if h < 3 else Q8b
                        K8 = K8a if h < 3 else K8b
                        pb = 32 * (h % 3)
                        nch = NCH * j + NCH   # s-chunks for this q-block

                        def s_mm(i):
                            fringe = i >= NCH * j
                            d = SC * i - TJ * j if fringe else 0
                            S = sps.tile([128, TJ], F32, tag="S")
                            nc.tensor.matmul(
                                S[:, d:TJ],
                                lhsT=K8[pb:pb + 32, :, ts(i, SC)],
                                rhs=Q8[pb:pb + 32, :, ds(j * TJ + d, TJ - d)],
                                start=True, stop=True,
                                perf_mode=DR,
                            )
                            return S, d, fringe

                        O = ops.tile([65, TJ], F32, tag="O")
                        pending = s_mm(0)
                        for i in range(nch):
                            S, d, fringe = pending
                            P = pp.tile([128, TJ], BF16, tag="P")
                            nc.scalar.activation(P[:, d:TJ], S[:, d:TJ],
                                                 AF.Exp, scale=SCALE)
                            if i + 1 < nch:
                                # software pipeline: S of chunk i+1 issues on
                                # PE before PV of chunk i (which waits on exp)
                                pending = s_mm(i + 1)
                            if fringe:
                                # diagonal window [d, d+128): keep iff p <= f-d
                                nc.gpsimd.affine_select(
                                    out=P[:, d:d + 128], in_=P[:, d:d + 128],
                                    pattern=[[1, 128]],
                                    compare_op=mybir.AluOpType.is_ge,
                                    fill=0.0, base=0, channel_multiplier=-1,
                                )
                            nc.tensor.matmul(
                                O[:, d:TJ],
                                lhsT=Vt[:, i, h, :],
                                rhs=P[:, d:TJ],
                                start=(i == 0), stop=(i == nch - 1),
                            )
                        # stage unnormalized attT (bf16) and transposed denom
                        po = (h % 2) * 64
                        bi = h // 2
                        nc.vector.tensor_copy(
                            attT[po:po + 64, bi, ts(j, TJ)], O[0:64, :]
                        )
                        dsb = rp.tile([1, TJ], F32, tag="dsb")
                        nc.vector.tensor_copy(dsb[:], O[64:65, :])
                        for q in range(NCH):
                            nc.tensor.transpose(
                                dT[:, h * NCH + q:h * NCH + q + 1],
                                dsb[0:1, ts(q, 128)], iden[0:1, 0:1],
                            )
                        # interleave previous block's output projection so the
                        # PE/DVE out-proj work fills ACT-bound gaps
                        if j > 0 and h < NCH:
                            out_proj(j - 1, h)
                    rT = rp.tile([128, NCH * H], F32, tag="rT")
                    nc.vector.reciprocal(rT[:], dT[:])
                    rTs[j] = rT
                # tail: last block's output projection
                for q in range(NCH):
                    out_proj(NJ - 1, q)

    nc.compile()
    return nc


def _prep_inputs(x, Wq, Wk, Wv, Wp, bp):
    """Host-side shard + layout prep. Returns per-core input maps."""
    bf = ml_dtypes.bfloat16
    f8 = ml_dtypes.float8_e4m3
    x = np.asarray(x, dtype=np.float32)

    def pack_qk8(W):
        # W [H, C, Dh] -> packed cols m = 192*pg + 96*i + p:
        # head = 3*pg + p//32, d = 32*i + p%32.
        M = np.empty((C, C), np.float32)
        for m in range(C):
            pg, mm = divmod(m, 192)
            i, p = divmod(mm, 96)
            M[:, m] = W[3 * pg + p // 32, :, 32 * i + p % 32]
        # -> [64, NCI, 2, C] with c = 128*ci + 64*i2 + p64
        return np.ascontiguousarray(
            M.reshape(NCI, 2, 64, C).transpose(2, 0, 1, 3)
        ).astype(f8)

    def pack_w(W):  # [H, C, Dh] -> [128, NCI, H*Dh]
        Whd = np.transpose(np.asarray(W, np.float32), (1, 0, 2)).reshape(C, H * DH)
        return np.ascontiguousarray(
            Whd.reshape(NCI, 128, H * DH).transpose(1, 0, 2)
        ).astype(bf)

    wq8, wk8 = pack_qk8(np.asarray(Wq, np.float32)), pack_qk8(np.asarray(Wk, np.float32))
    wv_p = pack_w(Wv)
    wp_p = np.ascontiguousarray(
        np.asarray(Wp, np.float32).reshape(NCI, 128, C).transpose(1, 0, 2)
    ).astype(bf)

    biasb = np.broadcast_to(np.asarray(bp, np.float32), (128, C)).copy()
    iden_np = np.ones((1, 1), dtype=np.float32)

    in_maps = []
    for b in range(B):
        xTb = x[b].T.reshape(NCI, 128, T)                  # [ci, c%128, t]
        xT = np.ascontiguousarray(xTb.transpose(1, 0, 2)).astype(bf)
        xT8 = np.ascontiguousarray(
            xTb.reshape(NCI, 2, 64, T).transpose(2, 0, 1, 3)
        ).astype(f8)
        in_maps.append({
            "xT": xT, "xT8": xT8, "wq8": wq8, "wk8": wk8, "wv": wv_p,
            "wp": wp_p, "biasb": biasb, "iden": iden_np,
        })
    return in_maps


_CACHE = {}


def kernel(x, Wq, Wk, Wv, Wp, bp):
    from concourse.bass_utils import run_bass_kernel_spmd

    if "nc" not in _CACHE:
        _CACHE["nc"] = build_kernel()
    nc = _CACHE["nc"]
    in_maps = _prep_inputs(x, Wq, Wk, Wv, Wp, bp)
    res = run_bass_kernel_spmd(nc, in_maps, list(range(NCORES)))
    out = np.stack([res.results[b]["y"] for b in range(B)], axis=0)
    return out.astype(np.float32)


# revision 8
# speedup vs baseline: 1.5240x; 1.5240x over previous
"""Multi-head causal attention (B=8, T=2048, C=384, H=6, Dh=64) on 8 TRN2 cores.

Sharding: data-parallel over batch — core b computes batch element b end to end
(no collectives).

Per-core kernel layout (all "T" means transposed, head-dim/channel on
partitions):
  xT   [128, 3, 2048]  bf16   c = 128*ci + p
  wq/wk[128, 3, 384]   bf16   packed Wq[h,c,d] -> [c, h*64+d]
  wv   [128, 3, 384]   bf16
  wp   [128, 3, 384]   bf16   Wp[c, e] -> [128, ci, e]
  bp   [128, 384]      f32    bias broadcast rows
  iden [6, 6]          f32    eye(6), transpose helper

Compute per core:
  QT/KT [hd, t] via matmul(lhsT=w chunk, rhs=xT)      (hd = h*64+d, 3 blocks)
  V_aug [s, 65] per (s-chunk, head), last col = 1     (stationary for PV)
  per q-block j (512 wide), head h (software-pipelined one S ahead):
    ST chunks [s=128, t<=512] = KT^T-slice @ QT-slice (K = d = 64)
    exp (ACT, scale=Dh^-0.5) -> P bf16; causal affine_select on diag window
    O_aug [65, d:512] += V_aug^T @ P[:, d:512]        (row 64 = softmax denom)
    denom rows staged into den6 [6, 512] per block
  block j's den6 is batch-transposed (4x [6,128] -> [128,6]) + recip'd lazily
  at the start of block j+1 so the PE never waits at block boundaries;
  out-proj of block j-1 interleaved into block j's head loop.

HAM clock-gate management: the TRN2 PE runs at 1.2 GHz until it has been
busy with zero gaps for a full 4096-cycle window (~3.4us), then 2.4 GHz.
Dependency-free "warm burst" matmul runs (dead writes into the U ring) are
injected at the attention-phase start and each block boundary to trip the
gate; the ACT exp table is preloaded in phase 1 so the first attention exp
doesn't open a pipeline gap.
"""

import numpy as np
import ml_dtypes

import concourse.bass as bass
import concourse.tile as tile
from concourse import bacc, mybir
from concourse.bass import ts, ds

F32 = mybir.dt.float32
BF16 = mybir.dt.bfloat16
AF = mybir.ActivationFunctionType

B, T, C = 8, 2048, 384
H, DH = 6, 64
SCALE = DH ** -0.5
NCORES = 8
TJ = 512            # q-block width
NJ = T // TJ        # 4 q-blocks
SC = 128            # s-chunk
NCI = C // 128      # 3 channel chunks
NCH = TJ // SC      # s-chunks per q-block (4)


def build_kernel():
    nc = bacc.Bacc("TRN2", target_bir_lowering=False, debug=False)

    xT_d = nc.dram_tensor("xT", [128, NCI, T], BF16, kind="ExternalInput").ap()
    wq_d = nc.dram_tensor("wq", [128, NCI, C], BF16, kind="ExternalInput").ap()
    wk_d = nc.dram_tensor("wk", [128, NCI, C], BF16, kind="ExternalInput").ap()
    wv_d = nc.dram_tensor("wv", [128, NCI, C], BF16, kind="ExternalInput").ap()
    wp_d = nc.dram_tensor("wp", [128, NCI, C], BF16, kind="ExternalInput").ap()
    biasb_d = nc.dram_tensor("biasb", [128, 384], F32, kind="ExternalInput").ap()
    iden_d = nc.dram_tensor("iden", [6, 6], F32, kind="ExternalInput").ap()
    y_d = nc.dram_tensor("y", [T, C], F32, kind="ExternalOutput").ap()

    with tile.TileContext(nc) as tc:
        with tc.tile_pool(name="const", bufs=1) as cpool:
            xT = cpool.tile([128, NCI, T], BF16)
            wq = cpool.tile([128, NCI, C], BF16)
            wk = cpool.tile([128, NCI, C], BF16)
            wv = cpool.tile([128, NCI, C], BF16)
            wp = cpool.tile([128, NCI, C], BF16)
            biasb = cpool.tile([128, 384], F32)
            iden = cpool.tile([6, 6], F32)
            QT = cpool.tile([128, NCI, T], BF16)
            KT = cpool.tile([128, NCI, T], BF16)
            attT = cpool.tile([128, NCI, T], BF16)
            Vt = cpool.tile([128, 16, H, 65], BF16)

            for ci in range(NCI):
                nc.sync.dma_start(xT[:, ci, :], xT_d[:, ci, :])
            nc.sync.dma_start(wq[:], wq_d[:])
            nc.sync.dma_start(wk[:], wk_d[:])
            nc.sync.dma_start(wv[:], wv_d[:])
            nc.sync.dma_start(wp[:], wp_d[:])
            nc.sync.dma_start(biasb[:], biasb_d[:])
            nc.sync.dma_start(iden[:], iden_d[:])
            # whole-tile memset (contiguous; strided memset fails ISA check);
            # V copies below overwrite cols 0:64, leaving col 64 == 1.0
            nc.gpsimd.memset(Vt[:], 1.0)
            # preload the ACT exp table during phase 1 so the first real exp
            # in the attention phase doesn't stall the pipeline ~1.3us
            scr = cpool.tile([1, 1], F32)
            nc.gpsimd.memset(scr[:], 0.0)
            nc.scalar.activation(scr[:], scr[:], AF.Exp, scale=1.0)

            # ---- phase 1: projections ----
            with tc.tile_pool(name="pqk", bufs=2, space="PSUM") as pqk, \
                 tc.tile_pool(name="pv", bufs=2, space="PSUM") as pvp:
                for dst, w in ((QT, wq), (KT, wk)):
                    for pi in range(NCI):
                        for tcn in range(T // 512):
                            ps = pqk.tile([128, 512], F32, tag="pqk")
                            for ci in range(NCI):
                                nc.tensor.matmul(
                                    ps[:],
                                    lhsT=w[:, ci, ts(pi, 128)],
                                    rhs=xT[:, ci, ts(tcn, 512)],
                                    start=(ci == 0), stop=(ci == NCI - 1),
                                )
                            nc.vector.tensor_copy(dst[:, pi, ts(tcn, 512)], ps[:])
                for si in range(16):
                    ps = pvp.tile([128, C], F32, tag="pv")
                    for ci in range(NCI):
                        nc.tensor.matmul(
                            ps[:],
                            lhsT=xT[:, ci, ts(si, 128)],
                            rhs=wv[:, ci, :],
                            start=(ci == 0), stop=(ci == NCI - 1),
                        )
                    nc.vector.tensor_copy(
                        Vt[:, si, :, 0:64],
                        ps[:].rearrange("p (h d) -> p h d", h=H),
                    )

            # ---- phase 2+3: attention + output projection ----
            with tc.tile_pool(name="sps", bufs=3, space="PSUM") as sps, \
                 tc.tile_pool(name="ops", bufs=2, space="PSUM") as ops, \
                 tc.tile_pool(name="dps", bufs=1, space="PSUM") as dps, \
                 tc.tile_pool(name="ups", bufs=2, space="PSUM") as ups, \
                 tc.tile_pool(name="pp", bufs=3) as pp, \
                 tc.tile_pool(name="rp", bufs=2) as rp, \
                 tc.tile_pool(name="yp", bufs=2) as yp:
                rTs = {}
                den6s = {}

                def warm_burst(n):
                    # dependency-free back-to-back matmuls (dead writes into
                    # the U ring, later overwritten) to trip the HAM gate
                    for _ in range(n):
                        Ub = ups.tile([128, C], F32, tag="U")
                        nc.tensor.matmul(
                            Ub[:], lhsT=xT[:, 0, 0:128], rhs=xT[:, 0, 0:C],
                            start=True, stop=True,
                        )

                def denom_finalize(jj):
                    # batch-transpose block jj's denominators + reciprocal
                    den6 = den6s.pop(jj)
                    dT = dps.tile([128, NCH, H], F32, tag="dT")
                    for q in range(NCH):
                        nc.tensor.transpose(
                            dT[:, q, :], den6[0:H, ts(q, 128)], iden[0:H, 0:H]
                        )
                    rT = rp.tile([128, NCH * H], F32, tag="rT")
                    nc.vector.reciprocal(
                        rT[:], dT[:].rearrange("p a b -> p (a b)"))
                    rTs[jj] = rT

                def out_proj(jj, q):
                    # Y[t, e] for t-chunk (jj, q): sum over heads of
                    # (attT_h^T @ wp_h) * recip_h[t], plus bias.
                    tb = NCH * jj + q
                    rT = rTs[jj]
                    Y = yp.tile([128, C], F32, tag="Y")
                    for h in range(H):
                        po = (h % 2) * 64
                        bi = h // 2
                        U = ups.tile([128, C], F32, tag="U")
                        nc.tensor.matmul(
                            U[:],
                            lhsT=attT[po:po + 64, bi, ts(tb, 128)],
                            rhs=wp[po:po + 64, bi, :],
                            start=True, stop=True,
                        )
                        sc = rT[:, q * H + h:q * H + h + 1]
                        nc.vector.scalar_tensor_tensor(
                            out=Y[:], in0=U[:], scalar=sc,
                            in1=(biasb[:] if h == 0 else Y[:]),
                            op0=mybir.AluOpType.mult,
                            op1=mybir.AluOpType.add,
                        )
                    nc.sync.dma_start(y_d[ts(tb, 128), :], Y[:])

                for j in range(NJ):
                    den6 = rp.tile([H, TJ], F32, tag="den6")
                    den6s[j] = den6
                    for h in range(H):
                        po = (h % 2) * 64     # partition offset inside hd-block
                        bi = h // 2           # hd block index
                        nch = NCH * j + NCH   # s-chunks for this q-block

                        def s_mm(i):
                            fringe = i >= NCH * j
                            d = SC * i - TJ * j if fringe else 0
                            S = sps.tile([128, TJ], F32, tag="S")
                            nc.tensor.matmul(
                                S[:, d:TJ],
                                lhsT=KT[po:po + 64, bi, ts(i, SC)],
                                rhs=QT[po:po + 64, bi, ds(j * TJ + d, TJ - d)],
                                start=True, stop=True,
                            )
                            return S, d, fringe

                        O = ops.tile([65, TJ], F32, tag="O")
                        pending = s_mm(0)
                        if j == 0 and h == 0:
                            # initial warm burst: runs while exp of the first
                            # chunk completes, and trips the HAM to 2.4 GHz
                            warm_burst(20)
                        for i in range(nch):
                            S, d, fringe = pending
                            P = pp.tile([128, TJ], BF16, tag="P")
                            nc.scalar.activation(P[:, d:TJ], S[:, d:TJ],
                                                 AF.Exp, scale=SCALE)
                            if i + 1 < nch:
                                # software pipeline: S of chunk i+1 issues on
                                # PE before PV of chunk i (which waits on exp)
                                pending = s_mm(i + 1)
                            if fringe:
                                # diagonal window [d, d+128): keep iff p <= f-d
                                nc.gpsimd.affine_select(
                                    out=P[:, d:d + 128], in_=P[:, d:d + 128],
                                    pattern=[[1, 128]],
                                    compare_op=mybir.AluOpType.is_ge,
                                    fill=0.0, base=0, channel_multiplier=-1,
                                )
                            nc.tensor.matmul(
                                O[:, d:TJ],
                                lhsT=Vt[:, i, h, :],
                                rhs=P[:, d:TJ],
                                start=(i == 0), stop=(i == nch - 1),
                            )
                        # stage unnormalized attT (bf16) and denom row
                        nc.vector.tensor_copy(
                            attT[po:po + 64, bi, ts(j, TJ)], O[0:64, :]
                        )
                        dsb = rp.tile([1, TJ], F32, tag="dsb")
                        nc.vector.tensor_copy(dsb[:], O[64:65, :])
                        # SBUF->SBUF DMA can target any partition; lands the
                        # denom row on den6 partition h for batch transposing
                        nc.sync.dma_start(den6[h:h + 1, :], dsb[:])
                        if j > 0 and h == 0:
                            # lazy: previous block's denom transposes + recip,
                            # emitted here so the PE stays fed at boundaries
                            denom_finalize(j - 1)
                        if j > 0 and h < NCH:
                            # interleave previous block's output projection
                            out_proj(j - 1, h)
                    if j < NJ - 1:
                        warm_burst(10)
                # tail: last block's output projection
                denom_finalize(NJ - 1)
                for q in range(NCH):
                    out_proj(NJ - 1, q)

    nc.compile()
    return nc


def _prep_inputs(x, Wq, Wk, Wv, Wp, bp):
    """Host-side shard + layout prep. Returns per-core input maps."""
    bf = ml_dtypes.bfloat16
    x = np.asarray(x, dtype=np.float32)

    def pack_w(W):  # [H, C, Dh] -> [128, NCI, H*Dh]
        Whd = np.transpose(np.asarray(W, np.float32), (1, 0, 2)).reshape(C, H * DH)
        return np.ascontiguousarray(
            Whd.reshape(NCI, 128, H * DH).transpose(1, 0, 2)
        ).astype(bf)

    wq_p, wk_p, wv_p = pack_w(Wq), pack_w(Wk), pack_w(Wv)
    wp_p = np.ascontiguousarray(
        np.asarray(Wp, np.float32).reshape(NCI, 128, C).transpose(1, 0, 2)
    ).astype(bf)

    biasb = np.broadcast_to(np.asarray(bp, np.float32), (128, C)).copy()
    iden_np = np.eye(6, dtype=np.float32)

    in_maps = []
    for b in range(B):
        xT = np.ascontiguousarray(
            x[b].T.reshape(NCI, 128, T).transpose(1, 0, 2)
        ).astype(bf)
        in_maps.append({
            "xT": xT, "wq": wq_p, "wk": wk_p, "wv": wv_p, "wp": wp_p,
            "biasb": biasb, "iden": iden_np,
        })
    return in_maps


_CACHE = {}


def kernel(x, Wq, Wk, Wv, Wp, bp):
    from concourse.bass_utils import run_bass_kernel_spmd

    if "nc" not in _CACHE:
        _CACHE["nc"] = build_kernel()
    nc = _CACHE["nc"]
    in_maps = _prep_inputs(x, Wq, Wk, Wv, Wp, bp)
    res = run_bass_kernel_spmd(nc, in_maps, list(range(NCORES)))
    out = np.stack([res.results[b]["y"] for b in range(B)], axis=0)
    return out.astype(np.float32)
